# revision 11
# baseline (speedup 1.0000x reference)
"""CosineVectorQuantizer forward on 8 trn2 NeuronCores.

Data-parallel: x sharded along batch (4096 rows/core), 8192x256 codebook
replicated. Per core: normalize codebook, transpose x and cb_norm to
K-major via PE, sim matmul on PE (fp32 PSUM accumulation), fused
DVE scan (copy d=sim-1 to SBUF + per-512-chunk max), InstMax/InstMaxIndex
for exact first-occurrence argmax, indirect-DMA codebook gather, per-row
projection/commit math. Loss mean is finished on host.
"""

import numpy as np

import concourse.bacc as bacc
import concourse.mybir as mybir
import concourse.tile as tile
from concourse.bass import IndirectOffsetOnAxis
from concourse.bass_utils import run_bass_kernel_spmd
from concourse.masks import make_identity

F32 = mybir.dt.float32
U32 = mybir.dt.uint32
AF = mybir.ActivationFunctionType
OP = mybir.AluOpType

N_CORES = 8
B, D, N = 32768, 256, 8192
BS = B // N_CORES          # 4096 rows per core
MT = BS // 128             # 32 m-tiles
NT = N // 128              # 64 codebook tiles
NCHUNK = 512               # psum bank width (fp32)
NCH = N // NCHUNK          # 16 chunks per row
BETA = 0.25

# float32r is tf32-like reduced precision (verifier demands pre-rounded
# inputs) — unusable here: top-2 sim gaps down to 1.6e-6 need fp32 accuracy.
MM_DT = mybir.dt.float32


def _build_nc(BS=BS, N=N, MT=None, NT=None, NCH=None):
    MT = BS // 128 if MT is None else MT
    NT = N // 128 if NT is None else NT
    NCH = N // NCHUNK if NCH is None else NCH
    nc = bacc.Bacc("TRN2", target_bir_lowering=False, debug=False)
    x_d = nc.dram_tensor("x", [BS, D], F32, kind="ExternalInput")
    e_d = nc.dram_tensor("embedding", [N, D], F32, kind="ExternalInput")
    xq_d = nc.dram_tensor("xq", [BS, D], F32, kind="ExternalOutput")
    idx_d = nc.dram_tensor("idx", [BS, 1], U32, kind="ExternalOutput")
    sc_d = nc.dram_tensor("sc", [BS, 1], F32, kind="ExternalOutput")
    cm_d = nc.dram_tensor("cm", [BS, 1], F32, kind="ExternalOutput")

    with tile.TileContext(nc) as tc:
        with (
            tc.tile_pool(name="persist", bufs=1) as pp,
            tc.tile_pool(name="io", bufs=2) as io,
            tc.tile_pool(name="dpp", bufs=2) as dpp,
            tc.tile_pool(name="small", bufs=3) as sp,
            tc.tile_pool(name="pst", bufs=2, space="PSUM") as pst,
            tc.tile_pool(name="psm", bufs=6, space="PSUM") as psm,
        ):
            ident = pp.tile([128, 128], F32, tag="ident")
            make_identity(nc, ident[:])
            cbT = [
                pp.tile([128, N], F32, tag=f"cbT{j}", name=f"cbT{j}")
                for j in range(2)
            ]
            xT = [
                pp.tile([128, BS], F32, tag=f"xT{j}", name=f"xT{j}")
                for j in range(2)
            ]
            xn2 = pp.tile([128, MT], F32, tag="xn2")

            # ---- phase 1: codebook load, normalize, transpose ----
            for t in range(NT):
                tsl = slice(t * 128, (t + 1) * 128)
                et = io.tile([128, D], F32, tag="et")
                nc.sync.dma_start(et[:], e_d[tsl, :])
                sq = io.tile([128, D], F32, tag="sq")
                nr = sp.tile([128, 1], F32, tag="nr")
                nc.scalar.activation(sq[:], et[:], AF.Square, accum_out=nr[:])
                nc.scalar.activation(nr[:], nr[:], AF.Sqrt)
                nc.vector.tensor_scalar_max(nr[:], nr[:], 1e-12)
                ri = sp.tile([128, 1], F32, tag="ri")
                nc.vector.reciprocal(ri[:], nr[:])
                cbn = io.tile([128, D], F32, tag="cbn")
                nc.vector.tensor_scalar_mul(cbn[:], et[:], ri[:])
                for j in range(2):
                    ps = pst.tile([128, 128], F32, tag="pst")
                    nc.tensor.transpose(ps[:], cbn[:, j * 128 : (j + 1) * 128], ident[:])
                    nc.scalar.copy(cbT[j][:, tsl], ps[:])

            # ---- phase 1b: x load, row norms, transpose ----
            for m in range(MT):
                msl = slice(m * 128, (m + 1) * 128)
                xt = io.tile([128, D], F32, tag="xt")
                nc.sync.dma_start(xt[:], x_d[msl, :])
                sqx = io.tile([128, D], F32, tag="sqx")
                nc.scalar.activation(sqx[:], xt[:], AF.Square, accum_out=xn2[:, m : m + 1])
                for j in range(2):
                    ps = pst.tile([128, 128], F32, tag="pst")
                    nc.tensor.transpose(ps[:], xt[:, j * 128 : (j + 1) * 128], ident[:])
                    nc.scalar.copy(xT[j][:, msl], ps[:])

            # ---- phase 2: sim matmul + argmax + finals per m-tile ----
            for m in range(MT):
                msl = slice(m * 128, (m + 1) * 128)
                dp = dpp.tile([128, N], F32, tag="dp")
                for c in range(NCH):
                    csl = slice(c * NCHUNK, (c + 1) * NCHUNK)
                    ps = psm.tile([128, NCHUNK], F32, tag="psm")
                    nc.tensor.matmul(
                        ps[:],
                        xT[0][:, msl].bitcast(MM_DT),
                        cbT[0][:, csl].bitcast(MM_DT),
                        start=True,
                        stop=False,
                    )
                    nc.tensor.matmul(
                        ps[:],
                        xT[1][:, msl].bitcast(MM_DT),
                        cbT[1][:, csl].bitcast(MM_DT),
                        start=False,
                        stop=True,
                    )
                    # d' = sim - 1: bit-exact negative of the reference's
                    # 1-sim distances, copied PSUM->SBUF on ACT.
                    nc.scalar.activation(dp[:, csl], ps[:], AF.Copy, bias=-1.0)
                rmax8 = sp.tile([128, 8], F32, tag="rmax8")
                nc.vector.max(rmax8[:], dp[:])
                idx8 = sp.tile([128, 8], U32, tag="idx8")
                nc.vector.max_index(idx8[:], rmax8[:], dp[:])

                cbv = io.tile([128, D], F32, tag="cbv")
                nc.gpsimd.indirect_dma_start(
                    out=cbv[:],
                    out_offset=None,
                    in_=e_d[:],
                    in_offset=IndirectOffsetOnAxis(ap=idx8[:, 0:1], axis=0),
                )
                xt2 = io.tile([128, D], F32, tag="xt2")
                nc.sync.dma_start(xt2[:], x_d[msl, :])

                scr = io.tile([128, D], F32, tag="scr")
                dotv = sp.tile([128, 1], F32, tag="dotv")
                nc.vector.tensor_mul(scr[:], xt2[:], cbv[:])
                nc.vector.tensor_reduce(
                    dotv[:], scr[:], axis=mybir.AxisListType.X, op=OP.add
                )
                scr2 = io.tile([128, D], F32, tag="scr2")
                nsq = sp.tile([128, 1], F32, tag="nsq")
                nc.scalar.activation(scr2[:], cbv[:], AF.Square, accum_out=nsq[:])
                den = sp.tile([128, 1], F32, tag="den")
                nc.scalar.activation(den[:], nsq[:], AF.Copy, bias=1e-8)
                rec = sp.tile([128, 1], F32, tag="rec")
                nc.vector.reciprocal(rec[:], den[:])
                sc = sp.tile([128, 1], F32, tag="sc")
                nc.vector.tensor_mul(sc[:], dotv[:], rec[:])

                proj = io.tile([128, D], F32, tag="proj")
                nc.vector.tensor_scalar_mul(proj[:], cbv[:], sc[:])
                dif = io.tile([128, D], F32, tag="dif")
                nc.vector.tensor_sub(dif[:], proj[:], xt2[:])
                xq = io.tile([128, D], F32, tag="xqt")
                nc.vector.tensor_add(xq[:], xt2[:], dif[:])

                scr3 = io.tile([128, D], F32, tag="scr3")
                pd = sp.tile([128, 1], F32, tag="pd")
                nc.vector.tensor_mul(scr3[:], proj[:], xt2[:])
                nc.vector.tensor_reduce(
                    pd[:], scr3[:], axis=mybir.AxisListType.X, op=OP.add
                )
                scr4 = io.tile([128, D], F32, tag="scr4")
                pn2 = sp.tile([128, 1], F32, tag="pn2")
                nc.scalar.activation(scr4[:], proj[:], AF.Square, accum_out=pn2[:])
                na = sp.tile([128, 1], F32, tag="na")
                nc.scalar.activation(na[:], pn2[:], AF.Sqrt)
                nc.vector.tensor_scalar_max(na[:], na[:], 1e-8)
                nb = sp.tile([128, 1], F32, tag="nb")
                nc.scalar.activation(nb[:], xn2[:, m : m + 1], AF.Sqrt)
                nc.vector.tensor_scalar_max(nb[:], nb[:], 1e-8)
                dn = sp.tile([128, 1], F32, tag="dn")
                nc.vector.tensor_mul(dn[:], na[:], nb[:])
                rc2 = sp.tile([128, 1], F32, tag="rc2")
                nc.vector.reciprocal(rc2[:], dn[:])
                cmv = sp.tile([128, 1], F32, tag="cmv")
                nc.vector.tensor_mul(cmv[:], pd[:], rc2[:])

                nc.sync.dma_start(xq_d[msl, :], xq[:])
                nc.sync.dma_start(idx_d[msl, :], idx8[:, 0:1])
                nc.sync.dma_start(sc_d[msl, :], sc[:])
                nc.sync.dma_start(cm_d[msl, :], cmv[:])

    nc.compile()
    return nc


_NC_CACHE = None


def _get_nc():
    global _NC_CACHE
    if _NC_CACHE is None:
        _NC_CACHE = _build_nc()
    return _NC_CACHE


def kernel(x, embedding, _trace=False, _trace_kwargs=None):
    nc = _get_nc()
    x = np.ascontiguousarray(x, dtype=np.float32)
    embedding = np.ascontiguousarray(embedding, dtype=np.float32)
    in_maps = [
        {"x": x[i * BS : (i + 1) * BS], "embedding": embedding}
        for i in range(N_CORES)
    ]
    res = run_bass_kernel_spmd(
        nc, in_maps, core_ids=list(range(N_CORES)), trace=_trace,
        **(_trace_kwargs or {}),
    )
    outs = res.results
    xq = np.concatenate([outs[i]["xq"] for i in range(N_CORES)], axis=0)
    idx = np.concatenate(
        [outs[i]["idx"][:, 0] for i in range(N_CORES)], axis=0
    ).astype(np.int32)
    sc = np.concatenate([outs[i]["sc"][:, 0] for i in range(N_CORES)], axis=0)
    cm = np.concatenate([outs[i]["cm"][:, 0] for i in range(N_CORES)], axis=0)
    loss = np.float32(BETA * np.mean(1.0 - cm.astype(np.float64)))
    if _trace:
        kernel._last_result = res
    return xq, loss, idx, sc
